# revision 1
# baseline (speedup 1.0000x reference)
"""GCN (2x GCNConv + LeakyReLU + Linear) on 8 Trainium2 NeuronCores.

Sharding: nodes partitioned contiguously across 8 cores (12500 real + 44
pad rows -> 12544 = 98 tiles of 128 per core). Edges assigned by
destination shard so the scatter-add stays local; self-loops included as
edges.

Key algebraic restructure: norm_e = dinv[src]*dinv[dst] factorizes, so
the per-edge weight disappears: tables store dinv-scaled rows
(x~ = dinv*x on host; A1~ = dinv*A1 on device), the aggregation is an
UNWEIGHTED segmented sum, and dinv[dst] is applied per output row.
Gathered edges land at [dst-node = partition, slot, feat] (the dma_gather
index order encodes the destination), so the whole segmented sum per
128-node tile is ONE vector-engine XY-reduce over a 4D access pattern --
no selection matrices, no per-chunk matmuls.

Layer 2 exploits linearity: aggregate first, then one fused [64, 4]
GEMM (W2 @ Wfc) done as N=512 matmuls on the transposed aggregate.

Per layer: local GEMM -> AllGather [12544,64] -> [100352,64] ->
bucketed dma_gather (4 sub-table buckets keep indices in int16 range) ->
per-tile reduce -> batched bias/LeakyReLU/dinv ops.
"""
import sys
import os

sys.path.insert(0, "/opt/trn_rl_repo")

import numpy as np

import concourse.bass as bass
import concourse.mybir as mybir
import concourse.tile as tile
import concourse.bacc as bacc
from concourse.bass_utils import run_bass_kernel_spmd
from concourse.library_config import mlp
from concourse.masks import make_identity

P = 128
NC = 8
NEG = 0.01
NBUCK = 4

F32 = mybir.dt.float32
I16 = mybir.dt.int16


class Cfg:
    def __init__(self, n_nodes, caps, s_tiles, nsh_tiles):
        self.n_nodes = n_nodes
        self.d_in = 128
        self.d_mid = 64
        self.d_out = 4
        self.caps = tuple(int(v) for v in caps)   # slots per (tile, bucket)
        self.s = s_tiles
        self.nsh_t = nsh_tiles
        self.nsh = nsh_tiles * P
        self.nreal = -(-n_nodes // NC)
        assert self.nreal < self.nsh, "need at least one pad row per shard"
        self.npad = NC * self.nsh
        self.buck = 2 * self.nsh                  # 8 shards / 4 buckets
        assert self.buck <= 32768
        assert nsh_tiles % s_tiles == 0
        self.ngrp = nsh_tiles // s_tiles
        self.zrow = self.nreal                    # in-bucket index of a zero row
        # per-group call size (chunks) and idx column bases
        self.szg = [sum(self.caps[g * s_tiles:(g + 1) * s_tiles])
                    for g in range(self.ngrp)]
        self.icols_g = [NBUCK * z * 8 for z in self.szg]   # int16 cols per group
        self.ibase = np.concatenate([[0], np.cumsum(self.icols_g)]).astype(int)
        self.icols = int(self.ibase[-1])


def build_nc(cfg: Cfg, outer_loop=1):
    c = cfg
    D = c.d_mid
    S = c.s
    nc = bacc.Bacc("TRN2", target_bir_lowering=False, debug=False,
                   num_devices=NC)
    t_xT = nc.dram_tensor("xT", [c.d_in, c.nsh], F32, kind="ExternalInput")
    t_w1 = nc.dram_tensor("w1", [c.d_in, D], F32, kind="ExternalInput")
    t_b1 = nc.dram_tensor("b1rep", [P, D], F32, kind="ExternalInput")
    t_wc = nc.dram_tensor("wcomb", [D, c.d_out], F32, kind="ExternalInput")
    t_bc = nc.dram_tensor("bcombT", [c.d_out, 1], F32, kind="ExternalInput")
    t_dinv = nc.dram_tensor("dinvc", [P, c.nsh_t], F32, kind="ExternalInput")
    t_idx = nc.dram_tensor("idx16", [P, c.icols], I16, kind="ExternalInput")
    t_out = nc.dram_tensor("out", [c.d_out, c.nsh], F32, kind="ExternalOutput")

    with tile.TileContext(nc) as tc:
        with (
            tc.tile_pool(name="const", bufs=1) as cp,
            tc.tile_pool(name="sb", bufs=2) as sbp,
            tc.tile_pool(name="ps_h", bufs=2, space="PSUM") as ps_h,
            tc.tile_pool(name="ps_t", bufs=2, space="PSUM") as ps_t,
            tc.tile_pool(name="ps_o", bufs=2, space="PSUM") as ps_o,
            tc.tile_pool(name="dram", bufs=1, space="DRAM") as dp,
        ):
            nc.gpsimd.load_library(mlp)

            ident = cp.tile([P, P], F32)
            make_identity(nc, ident[:])
            w1_sb = cp.tile([c.d_in, D], F32)
            nc.sync.dma_start(w1_sb[:], t_w1[:])
            b1_sb = cp.tile([P, D], F32)
            nc.sync.dma_start(b1_sb[:], t_b1[:])
            wc_sb = cp.tile([D, c.d_out], F32)
            nc.sync.dma_start(wc_sb[:], t_wc[:])
            bcT_sb = cp.tile([c.d_out, 1], F32)
            nc.sync.dma_start(bcT_sb[:], t_bc[:])
            dinv_sb = cp.tile([P, c.nsh_t], F32)
            nc.sync.dma_start(dinv_sb[:], t_dinv[:])

            h_shard = dp.tile([c.nsh, D], F32)
            a_shard = dp.tile([c.nsh, D], F32)
            h_fulls = [dp.tile([c.npad, D], F32, addr_space="Shared",
                               name=f"h_full{r}") for r in range(outer_loop)]
            a_fulls = [dp.tile([c.npad, D], F32, addr_space="Shared",
                               name=f"a_full{r}") for r in range(outer_loop)]

            # ---- phase A: h_shard = (dinv*x) @ W1, row-major for gathers ----
            with tc.tile_pool(name="pa", bufs=1) as pa:
                xT_sb = pa.tile([c.d_in, c.nsh], F32)
                nc.sync.dma_start(xT_sb[:], t_xT[:])
                hstage = pa.tile([P, c.nsh_t * D], F32)
                for i in range(c.nsh_t):
                    ph = ps_h.tile([P, D], F32, space="PSUM", tag="ph")
                    nc.tensor.matmul(out=ph[:],
                                     lhsT=xT_sb[:, i * P:(i + 1) * P],
                                     rhs=w1_sb[:], start=True, stop=True)
                    nc.vector.tensor_copy(hstage[:, i * D:(i + 1) * D], ph[:])
                nc.sync.dma_start(
                    h_shard[:].rearrange("(t p) f -> p t f", p=P),
                    hstage[:].rearrange("p (t f) -> p t f", f=D))

            def msg_pass(table, layer, gxp, z2T=None):
                for g in range(c.ngrp):
                    szg = c.szg[g]
                    ib = sbp.tile([P, c.icols_g[g]], I16, tag="ib")
                    nc.sync.dma_start(
                        ib[:], t_idx[:, int(c.ibase[g]):int(c.ibase[g + 1])])
                    gb = gxp.tile([P, NBUCK * szg * D], F32, tag="gb")
                    for b in range(NBUCK):
                        nc.gpsimd.dma_gather(
                            gb[:, b * szg * D:(b + 1) * szg * D]
                              .rearrange("p (ch f) -> p ch f", ch=szg),
                            table[b * c.buck:(b + 1) * c.buck, :],
                            ib[:, b * szg * 8:(b + 1) * szg * 8],
                            szg * P, szg * P, D, single_packet=False,
                        )
                    stage = sbp.tile([P, S * D], F32, tag="stage")
                    off = 0
                    for ti in range(S):
                        cap = c.caps[g * S + ti]
                        ap4 = (gb[:]
                               .rearrange("p (b ch f) -> p b ch f", b=NBUCK,
                                          ch=szg)[:, :, off:off + cap, :]
                               .rearrange("p b s f -> p f b s"))
                        nc.vector.reduce_sum(
                            out=stage[:, ti * D:(ti + 1) * D]
                                .unsqueeze(2).unsqueeze(3),
                            in_=ap4, axis=mybir.AxisListType.XY)
                        off += cap
                    dv = dinv_sb[:, g * S:g * S + S].unsqueeze(2) \
                        .broadcast_to([P, S, D])
                    st3 = stage[:].rearrange("p (t f) -> p t f", f=D)
                    nc.vector.tensor_tensor(out=st3, in0=st3, in1=dv,
                                            op=mybir.AluOpType.mult)
                    if layer == 1:
                        b13 = b1_sb[:].unsqueeze(1).broadcast_to([P, S, D])
                        nc.vector.tensor_tensor(out=st3, in0=st3, in1=b13,
                                                op=mybir.AluOpType.add)
                        asc = sbp.tile([P, S * D], F32, tag="asc")
                        nc.scalar.activation(
                            asc[:], stage[:],
                            mybir.ActivationFunctionType.Copy, scale=NEG)
                        nc.vector.tensor_tensor(out=stage[:], in0=stage[:],
                                                in1=asc[:],
                                                op=mybir.AluOpType.max)
                        nc.vector.tensor_tensor(out=st3, in0=st3, in1=dv,
                                                op=mybir.AluOpType.mult)
                        nc.sync.dma_start(
                            a_shard[g * S * P:(g + 1) * S * P, :]
                                .rearrange("(t p) f -> p t f", p=P),
                            st3)
                    else:
                        for ti in range(S):
                            t = g * S + ti
                            pt = ps_t.tile([D, P], F32, space="PSUM", tag="pt")
                            nc.tensor.transpose(
                                out=pt[:], in_=stage[:, ti * D:(ti + 1) * D],
                                identity=ident[:])
                            nc.vector.tensor_copy(
                                z2T[:, t * P:(t + 1) * P], pt[:])

            for _r in range(outer_loop):
              nc.gpsimd.collective_compute(
                  "AllGather", mybir.AluOpType.bypass,
                  replica_groups=[list(range(NC))],
                  ins=[h_shard.opt()], outs=[h_fulls[_r].opt()],
              )
              with tc.tile_pool(name=f"gx1_{_r}", bufs=1) as gx1:
                msg_pass(h_fulls[_r], 1, gx1)

              nc.gpsimd.collective_compute(
                  "AllGather", mybir.AluOpType.bypass,
                  replica_groups=[list(range(NC))],
                  ins=[a_shard.opt()], outs=[a_fulls[_r].opt()],
              )

              with (
                  tc.tile_pool(name=f"gx2_{_r}", bufs=1) as gx2,
                  tc.tile_pool(name=f"zt_{_r}", bufs=1) as ztp,
              ):
                z2T = ztp.tile([D, c.nsh], F32, name=f"z2T{_r}")
                msg_pass(a_fulls[_r], 2, gx2, z2T)
                # fused FC: out^T = Wcomb^T @ z2T + bcomb
                NB = 512
                for blk in range(-(-c.nsh // NB)):
                    n0 = blk * NB
                    n1 = min(c.nsh, n0 + NB)
                    pf = ps_o.tile([c.d_out, NB], F32, space="PSUM", tag="pf")
                    nc.tensor.matmul(out=pf[:, :n1 - n0], lhsT=wc_sb[:],
                                     rhs=z2T[:, n0:n1], start=True, stop=True)
                    ob = sbp.tile([c.d_out, NB], F32, tag="ob")
                    nc.vector.tensor_tensor(
                        out=ob[:, :n1 - n0], in0=pf[:, :n1 - n0],
                        in1=bcT_sb[:].broadcast_to([c.d_out, n1 - n0]),
                        op=mybir.AluOpType.add)
                    nc.sync.dma_start(t_out[:, n0:n1], ob[:, :n1 - n0])

    nc.compile()
    return nc


def host_prep(x, edge_index, W1, b1, W2, b2, Wfc, bfc, cfg: Cfg):
    c = cfg
    n = c.n_nodes
    S = c.s
    src = np.asarray(edge_index[0], dtype=np.int64)
    dst = np.asarray(edge_index[1], dtype=np.int64)

    deg = np.bincount(dst, minlength=n).astype(np.float64) + 1.0
    dinv = (1.0 / np.sqrt(deg)).astype(np.float32)

    esrc = np.concatenate([src, np.arange(n, dtype=np.int64)])
    edst = np.concatenate([dst, np.arange(n, dtype=np.int64)])

    nreal = c.nreal
    core = edst // nreal
    loc = edst - core * nreal
    tl = loc // P
    d_loc = loc % P
    gsrc = (esrc // nreal) * c.nsh + (esrc % nreal)
    buck = gsrc // c.buck
    inb = (gsrc - buck * c.buck).astype(np.int16)

    # slot rank within each (core, tile, bucket, dst-node) cell
    cell = ((core * c.nsh_t + tl) * NBUCK + buck) * P + d_loc
    order = np.argsort(cell, kind="stable")
    cell_s = cell[order]
    nbins = NC * c.nsh_t * NBUCK * P
    start = np.searchsorted(cell_s, np.arange(nbins))
    rank = np.arange(len(cell_s)) - start[cell_s]
    slot = np.empty(len(cell), np.int64)
    slot[order] = rank

    # edge -> idx16 position
    caps = np.asarray(c.caps, np.int64)
    grp_off = np.concatenate([[0], np.cumsum(caps)])      # global chunk cumsum
    g_of_t = np.arange(c.nsh_t) // S
    grp_start_chunk = grp_off[g_of_t * S]                 # chunks before group
    tioff = grp_off[:c.nsh_t] - grp_start_chunk           # chunk offset in group
    szg_arr = np.asarray(c.szg, np.int64)
    ibase = np.asarray(c.ibase, np.int64)

    g_e = tl // S
    i_in_call = (tioff[tl] + slot) * P + d_loc
    col = ibase[g_e] + buck * szg_arr[g_e] * 8 + i_in_call // 16
    row = i_in_call % 16

    idx16 = np.full((NC, 16, c.icols), c.zrow, np.int16)
    idx16[core, row, col] = inb
    idx16 = np.tile(idx16, (1, 8, 1))                     # [NC, 128, icols]

    # dinv-scaled, padded, sharded, transposed x
    x = np.asarray(x, dtype=np.float32) * dinv[:, None]
    xT = np.zeros((NC, c.d_in, c.nsh), np.float32)
    dinvc = np.zeros((NC, P, c.nsh_t), np.float32)
    for ci in range(NC):
        r0 = ci * nreal
        r1 = min(n, r0 + nreal)
        xT[ci, :, :r1 - r0] = x[r0:r1].T
        dv = np.zeros(c.nsh, np.float32)
        dv[:r1 - r0] = dinv[r0:r1]
        dinvc[ci] = dv.reshape(c.nsh_t, P).T

    W1 = np.ascontiguousarray(np.asarray(W1, np.float32))
    b1rep = np.tile(np.asarray(b1, np.float32)[None, :], (P, 1))
    wcomb = np.ascontiguousarray(
        np.asarray(W2, np.float32) @ np.asarray(Wfc, np.float32))
    bcomb = (np.asarray(b2, np.float32) @ np.asarray(Wfc, np.float32)
             + np.asarray(bfc, np.float32))
    bcombT = np.ascontiguousarray(bcomb[:, None])

    in_maps = []
    for ci in range(NC):
        in_maps.append({
            "xT": np.ascontiguousarray(xT[ci]),
            "w1": W1, "b1rep": b1rep, "wcomb": wcomb, "bcombT": bcombT,
            "dinvc": np.ascontiguousarray(dinvc[ci]),
            "idx16": np.ascontiguousarray(idx16[ci]),
        })
    return in_maps


_NC_CACHE = {}
LAST_RESULTS = None


def make_cfg(x, edge_index):
    n = x.shape[0]
    nreal = -(-n // NC)
    nsh_t = nreal // P + 1          # always >= 1 pad row per shard
    s = max(d for d in range(1, nsh_t + 1) if nsh_t % d == 0 and d <= 7)

    src = np.asarray(edge_index[0], dtype=np.int64)
    dst = np.asarray(edge_index[1], dtype=np.int64)
    esrc = np.concatenate([src, np.arange(n, dtype=np.int64)])
    edst = np.concatenate([dst, np.arange(n, dtype=np.int64)])
    core = edst // nreal
    loc = edst - core * nreal
    tl = loc // P
    d_loc = loc % P
    nsh = nsh_t * P
    gsrc = (esrc // nreal) * nsh + (esrc % nreal)
    buck = gsrc // (2 * nsh)
    cell = ((core * nsh_t + tl) * NBUCK + buck) * P + d_loc
    cnt = np.bincount(cell, minlength=NC * nsh_t * NBUCK * P)
    caps = cnt.reshape(NC, nsh_t, NBUCK, P).max(axis=(0, 2, 3))
    caps = np.maximum(caps, 1)
    return Cfg(n, caps, s, nsh_t)


def kernel(x, edge_index, W1, b1, W2, b2, Wfc, bfc):
    x = np.asarray(x)
    edge_index = np.asarray(edge_index)
    n = x.shape[0]

    cfg = make_cfg(x, edge_index)
    key = (n, cfg.nsh_t, cfg.s, cfg.caps)
    if key not in _NC_CACHE:
        _NC_CACHE[key] = build_nc(cfg)
    nc = _NC_CACHE[key]

    in_maps = host_prep(x, edge_index, W1, b1, W2, b2, Wfc, bfc, cfg)
    res = run_bass_kernel_spmd(nc, in_maps, core_ids=list(range(NC)))
    global LAST_RESULTS
    LAST_RESULTS = res

    outs = []
    left = n
    for ci in range(NC):
        take = min(cfg.nreal, left)
        outs.append(res.results[ci]["out"].T[:take])
        left -= take
    return np.ascontiguousarray(
        np.concatenate(outs, axis=0)).astype(np.float32)



# revision 2
# speedup vs baseline: 1.0496x; 1.0496x over previous
"""GCN (2x GCNConv + LeakyReLU + Linear) on 8 Trainium2 NeuronCores — v2.

Same sharding/data layout as the baseline kernel (nodes contiguous across 8
cores, edges by destination shard, self-loops as edges, dinv factorization,
unweighted segmented-sum via per-tile XY-reduce over a [p, f, bucket, slot]
access pattern), with the structural fixes found by profiling:

1. dma_gather calls are split to <= MAXCH chunks (~6k indices): calls above
   ~8k indices hit a severe slow path (~12x).
2. Reduce outputs land in one per-layer [128, 98*64] stage; the dinv/bias/
   LeakyReLU post-processing runs as ~5 whole-layer instructions instead of
   ~7 per group (per-instruction overhead on this runtime is 10s of us).
3. Layer-2 tail (z2 @ (W2@Wfc) + b) is computed with broadcast-multiply +
   axis-X reduce on the vector engine instead of 98 PE transposes + 25
   matmuls; dinv and bias are folded in afterwards (dinv commutes with the
   linear map).
"""
import sys
import os

sys.path.insert(0, "/opt/trn_rl_repo")

import numpy as np

import concourse.bass as bass
import concourse.mybir as mybir
import concourse.tile as tile
import concourse.bacc as bacc
from concourse.bass_utils import run_bass_kernel_spmd
from concourse.library_config import mlp

P = 128
NC = 8
NEG = 0.01
NBUCK = 4
MAXCH = int(os.environ.get("V2_MAXCH", "47"))
NQ = int(os.environ.get("V2_NQ", "1"))
NORED = bool(int(os.environ.get("V2_NORED", "0")))
V2_S = int(os.environ.get("V2_S", "0"))
V2_BUFS = int(os.environ.get("V2_BUFS", "1"))
NEGPAD = bool(int(os.environ.get("V2_NEGPAD", "0")))

F32 = mybir.dt.float32
I16 = mybir.dt.int16


class Cfg:
    def __init__(self, n_nodes, caps, s_tiles, nsh_tiles):
        self.n_nodes = n_nodes
        self.d_in = 128
        self.d_mid = 64
        self.d_out = 4
        self.caps = tuple(int(v) for v in caps)   # slots per (tile, bucket)
        self.s = s_tiles
        self.nsh_t = nsh_tiles
        self.nsh = nsh_tiles * P
        self.nreal = -(-n_nodes // NC)
        assert self.nreal < self.nsh, "need at least one pad row per shard"
        self.npad = NC * self.nsh
        self.buck = 2 * self.nsh                  # 8 shards / 4 buckets
        assert self.buck <= 32768
        assert nsh_tiles % s_tiles == 0
        self.ngrp = nsh_tiles // s_tiles
        self.zrow = self.nreal                    # in-bucket index of a zero row
        # per-group call size (chunks) and idx column bases
        self.szg = [sum(self.caps[g * s_tiles:(g + 1) * s_tiles])
                    for g in range(self.ngrp)]
        self.icols_g = [NBUCK * z * 8 for z in self.szg]   # int16 cols per group
        self.ibase = np.concatenate([[0], np.cumsum(self.icols_g)]).astype(int)
        self.icols = int(self.ibase[-1])


def build_nc(cfg: Cfg, outer_loop=1):
    c = cfg
    D = c.d_mid
    S = c.s
    T = c.nsh_t
    nc = bacc.Bacc("TRN2", target_bir_lowering=False, debug=False,
                   num_devices=NC, num_swdge_queues=NQ)
    t_xT = nc.dram_tensor("xT", [c.d_in, c.nsh], F32, kind="ExternalInput")
    t_w1 = nc.dram_tensor("w1", [c.d_in, D], F32, kind="ExternalInput")
    t_b1 = nc.dram_tensor("b1rep", [P, D], F32, kind="ExternalInput")
    t_wc = nc.dram_tensor("wcombrep", [P, c.d_out * D], F32,
                          kind="ExternalInput")
    t_bc = nc.dram_tensor("bcrep", [P, c.d_out], F32, kind="ExternalInput")
    t_dinv = nc.dram_tensor("dinvc", [P, T], F32, kind="ExternalInput")
    t_idx = nc.dram_tensor("idx16", [P, c.icols], I16, kind="ExternalInput")
    t_out = nc.dram_tensor("out", [c.d_out, c.nsh], F32, kind="ExternalOutput")

    with tile.TileContext(nc) as tc:
        with (
            tc.tile_pool(name="const", bufs=1) as cp,
            tc.tile_pool(name="stg", bufs=1) as stp,
            tc.tile_pool(name="sb", bufs=2) as sbp,
            tc.tile_pool(name="ps_h", bufs=2, space="PSUM") as ps_h,
            tc.tile_pool(name="dram", bufs=1, space="DRAM") as dp,
        ):
            nc.gpsimd.load_library(mlp)

            w1_sb = cp.tile([c.d_in, D], F32)
            nc.sync.dma_start(w1_sb[:], t_w1[:])
            b1_sb = cp.tile([P, D], F32)
            nc.sync.dma_start(b1_sb[:], t_b1[:])
            wc_sb = cp.tile([P, c.d_out * D], F32)
            nc.sync.dma_start(wc_sb[:], t_wc[:])
            bc_sb = cp.tile([P, c.d_out], F32)
            nc.sync.dma_start(bc_sb[:], t_bc[:])
            dinv_sb = cp.tile([P, T], F32)
            nc.sync.dma_start(dinv_sb[:], t_dinv[:])

            h_shard = dp.tile([c.nsh, D], F32)
            a_shard = dp.tile([c.nsh, D], F32)
            h_fulls = [dp.tile([c.npad, D], F32, addr_space="Shared",
                               name=f"h_full{r}") for r in range(outer_loop)]
            a_fulls = [dp.tile([c.npad, D], F32, addr_space="Shared",
                               name=f"a_full{r}") for r in range(outer_loop)]

            # ---- phase A: h_shard = (dinv*x) @ W1, row-major for gathers ----
            with tc.tile_pool(name="pa", bufs=1) as pa:
                xT_sb = pa.tile([c.d_in, c.nsh], F32)
                nc.sync.dma_start(xT_sb[:], t_xT[:])
                hstage = pa.tile([P, T * D], F32)
                for i in range(T):
                    ph = ps_h.tile([P, D], F32, space="PSUM", tag="ph")
                    nc.tensor.matmul(out=ph[:],
                                     lhsT=xT_sb[:, i * P:(i + 1) * P],
                                     rhs=w1_sb[:], start=True, stop=True)
                    nc.vector.tensor_copy(hstage[:, i * D:(i + 1) * D], ph[:])
                nc.sync.dma_start(
                    h_shard[:].rearrange("(t p) f -> p t f", p=P),
                    hstage[:].rearrange("p (t f) -> p t f", f=D))

            qctr = [0]

            def msg_pass(table, layer, gxp, stage):
                # gather + per-tile segmented sums into `stage` [P, T*D]
                st3 = stage[:].rearrange("p (t f) -> p t f", f=D)
                for g in range(c.ngrp):
                    szg = c.szg[g]
                    ib = sbp.tile([P, c.icols_g[g]], I16, tag="ib")
                    nc.sync.dma_start(
                        ib[:], t_idx[:, int(c.ibase[g]):int(c.ibase[g + 1])])
                    gb = gxp.tile([P, NBUCK * szg * D], F32, tag="gb")
                    if NEGPAD:
                        nc.vector.memset(gb[:], 0.0)
                    # range-outer / bucket-inner call order: a chunk range's
                    # four bucket slices complete together, so the reduces
                    # for its tiles can overlap the next range's gathers.
                    for c0 in range(0, szg, MAXCH):
                        c1 = min(szg, c0 + MAXCH)
                        for b in range(NBUCK):
                            nc.gpsimd.dma_gather(
                                gb[:, (b * szg + c0) * D:(b * szg + c1) * D]
                                  .rearrange("p (ch f) -> p ch f", ch=c1 - c0),
                                table[b * c.buck:(b + 1) * c.buck, :],
                                ib[:, (b * szg + c0) * 8:(b * szg + c1) * 8],
                                (c1 - c0) * P, (c1 - c0) * P, D,
                                single_packet=False,
                                queue_num=qctr[0] % NQ,
                            )
                            qctr[0] += 1
                    if NORED:
                        nc.vector.memset(stage[:, g * S * D:(g + 1) * S * D],
                                         0.0)
                        continue
                    off = 0
                    for ti in range(S):
                        t = g * S + ti
                        cap = c.caps[t]
                        ap4 = (gb[:]
                               .rearrange("p (b ch f) -> p b ch f", b=NBUCK,
                                          ch=szg)[:, :, off:off + cap, :]
                               .rearrange("p b s f -> p f b s"))
                        nc.vector.reduce_sum(
                            out=stage[:, t * D:(t + 1) * D]
                                .unsqueeze(2).unsqueeze(3),
                            in_=ap4, axis=mybir.AxisListType.XY)
                        off += cap

                dv = dinv_sb[:].unsqueeze(2).broadcast_to([P, T, D])
                if layer == 1:
                    # a = leaky(agg*dinv + b1); table2 rows = dinv*a
                    nc.vector.tensor_tensor(out=st3, in0=st3, in1=dv,
                                            op=mybir.AluOpType.mult)
                    b13 = b1_sb[:].unsqueeze(1).broadcast_to([P, T, D])
                    nc.vector.tensor_tensor(out=st3, in0=st3, in1=b13,
                                            op=mybir.AluOpType.add)
                    asc = stp.tile([P, T * D], F32, tag="asc")
                    nc.scalar.activation(
                        asc[:], stage[:],
                        mybir.ActivationFunctionType.Copy, scale=NEG)
                    nc.vector.tensor_tensor(out=stage[:], in0=stage[:],
                                            in1=asc[:],
                                            op=mybir.AluOpType.max)
                    nc.vector.tensor_tensor(out=st3, in0=st3, in1=dv,
                                            op=mybir.AluOpType.mult)
                    nc.sync.dma_start(
                        a_shard[:].rearrange("(t p) f -> p t f", p=P),
                        st3)

            for _r in range(outer_loop):
              nc.gpsimd.collective_compute(
                  "AllGather", mybir.AluOpType.bypass,
                  replica_groups=[list(range(NC))],
                  ins=[h_shard.opt()], outs=[h_fulls[_r].opt()],
              )
              with tc.tile_pool(name=f"gx1_{_r}", bufs=V2_BUFS) as gx1:
                stage1 = gx1.tile([P, T * D], F32, name=f"st1_{_r}")
                msg_pass(h_fulls[_r], 1, gx1, stage1)

              nc.gpsimd.collective_compute(
                  "AllGather", mybir.AluOpType.bypass,
                  replica_groups=[list(range(NC))],
                  ins=[a_shard.opt()], outs=[a_fulls[_r].opt()],
              )

              with tc.tile_pool(name=f"gx2_{_r}", bufs=V2_BUFS) as gx2:
                stage2 = gx2.tile([P, T * D], F32, name=f"st2_{_r}")
                msg_pass(a_fulls[_r], 2, gx2, stage2)
                # FC tail: out[o, (t p)] = dinv[n] * (agg2[n] @ Wc)[o] + bc[o]
                st3 = stage2[:].rearrange("p (t f) -> p t f", f=D)
                TH = T // 2
                for h in range(2):
                    t0 = h * TH
                    tmp = gx2.tile([P, TH * c.d_out * D], F32, tag="gb")
                    tmp4 = tmp[:].rearrange("p (t o f) -> p t o f",
                                            o=c.d_out, f=D)
                    in0 = st3[:, t0:t0 + TH, :].unsqueeze(2) \
                        .broadcast_to([P, TH, c.d_out, D])
                    in1 = wc_sb[:].rearrange("p (o f) -> p o f", o=c.d_out) \
                        .unsqueeze(1).broadcast_to([P, TH, c.d_out, D])
                    nc.vector.tensor_tensor(out=tmp4, in0=in0, in1=in1,
                                            op=mybir.AluOpType.mult)
                    red = gx2.tile([P, TH * c.d_out], F32, tag="fcred")
                    red3 = red[:].rearrange("p (t o) -> p t o", o=c.d_out)
                    nc.vector.reduce_sum(out=red3.unsqueeze(3), in_=tmp4,
                                         axis=mybir.AxisListType.X)
                    dv3 = dinv_sb[:, t0:t0 + TH].unsqueeze(2) \
                        .broadcast_to([P, TH, c.d_out])
                    nc.vector.tensor_tensor(out=red3, in0=red3, in1=dv3,
                                            op=mybir.AluOpType.mult)
                    bc3 = bc_sb[:].unsqueeze(1).broadcast_to([P, TH, c.d_out])
                    nc.vector.tensor_tensor(out=red3, in0=red3, in1=bc3,
                                            op=mybir.AluOpType.add)
                    for o in range(c.d_out):
                        nc.sync.dma_start(
                            t_out[o:o + 1, t0 * P:(t0 + TH) * P]
                            .rearrange("one (t p) -> p (one t)", p=P),
                            red3[:, :, o])

    nc.compile()
    return nc


def node_perm(edge_index, n):
    """Per-core degree-sorted local positions: newloc[g] for global node g.

    Sorting each shard's nodes by in-degree makes per-tile slot caps tight
    (tiles hold nearly-equal-degree nodes), cutting gather descriptors.
    """
    dst = np.asarray(edge_index[1], dtype=np.int64)
    deg = np.bincount(np.concatenate([dst, np.arange(n, dtype=np.int64)]),
                      minlength=n)
    nreal = -(-n // NC)
    newloc = np.empty(n, np.int64)
    for ci in range(NC):
        lo, hi = ci * nreal, min(n, (ci + 1) * nreal)
        order = np.argsort(-deg[lo:hi], kind="stable")
        newloc[lo:hi][order] = np.arange(hi - lo)
    return newloc


def host_prep(x, edge_index, W1, b1, W2, b2, Wfc, bfc, cfg: Cfg):
    c = cfg
    n = c.n_nodes
    S = c.s
    src = np.asarray(edge_index[0], dtype=np.int64)
    dst = np.asarray(edge_index[1], dtype=np.int64)

    deg = np.bincount(dst, minlength=n).astype(np.float64) + 1.0
    dinv = (1.0 / np.sqrt(deg)).astype(np.float32)

    esrc = np.concatenate([src, np.arange(n, dtype=np.int64)])
    edst = np.concatenate([dst, np.arange(n, dtype=np.int64)])

    nreal = c.nreal
    newloc = node_perm(edge_index, n)
    core = edst // nreal
    loc = newloc[edst]
    tl = loc // P
    d_loc = loc % P
    gsrc = (esrc // nreal) * c.nsh + newloc[esrc]
    buck = gsrc // c.buck
    inb = (gsrc - buck * c.buck).astype(np.int16)

    # slot rank within each (core, tile, bucket, dst-node) cell
    cell = ((core * c.nsh_t + tl) * NBUCK + buck) * P + d_loc
    order = np.argsort(cell, kind="stable")
    cell_s = cell[order]
    nbins = NC * c.nsh_t * NBUCK * P
    start = np.searchsorted(cell_s, np.arange(nbins))
    rank = np.arange(len(cell_s)) - start[cell_s]
    slot = np.empty(len(cell), np.int64)
    slot[order] = rank

    # edge -> idx16 position
    caps = np.asarray(c.caps, np.int64)
    grp_off = np.concatenate([[0], np.cumsum(caps)])      # global chunk cumsum
    g_of_t = np.arange(c.nsh_t) // S
    grp_start_chunk = grp_off[g_of_t * S]                 # chunks before group
    tioff = grp_off[:c.nsh_t] - grp_start_chunk           # chunk offset in group
    szg_arr = np.asarray(c.szg, np.int64)
    ibase = np.asarray(c.ibase, np.int64)

    g_e = tl // S
    i_in_call = (tioff[tl] + slot) * P + d_loc
    col = ibase[g_e] + buck * szg_arr[g_e] * 8 + i_in_call // 16
    row = i_in_call % 16

    keep = slot < caps[tl]
    if NEGPAD:
        # pads skip descriptor generation; buckets 1-3 only (a -1 from
        # bucket 0's base would address before the table if it *were* read)
        idx16 = np.full((NC, 16, c.icols), -1, np.int16)
        for g in range(c.ngrp):
            b0 = int(ibase[g])
            idx16[:, :, b0:b0 + c.szg[g] * 8] = c.zrow
    else:
        idx16 = np.full((NC, 16, c.icols), c.zrow, np.int16)
    idx16[core[keep], row[keep], col[keep]] = inb[keep]
    idx16 = np.tile(idx16, (1, 8, 1))                     # [NC, 128, icols]

    # dinv-scaled, padded, sharded, transposed x (degree-sorted node order)
    x = np.asarray(x, dtype=np.float32) * dinv[:, None]
    xT = np.zeros((NC, c.d_in, c.nsh), np.float32)
    dinvc = np.zeros((NC, P, c.nsh_t), np.float32)
    for ci in range(NC):
        r0 = ci * nreal
        r1 = min(n, r0 + nreal)
        cols = newloc[r0:r1]
        xT[ci][:, cols] = x[r0:r1].T
        dv = np.zeros(c.nsh, np.float32)
        dv[cols] = dinv[r0:r1]
        dinvc[ci] = dv.reshape(c.nsh_t, P).T

    W1 = np.ascontiguousarray(np.asarray(W1, np.float32))
    b1rep = np.tile(np.asarray(b1, np.float32)[None, :], (P, 1))
    wcomb = (np.asarray(W2, np.float32) @ np.asarray(Wfc, np.float32))
    wcombrep = np.ascontiguousarray(
        np.tile(wcomb.T.reshape(1, -1), (P, 1)))      # [P, (o f)]
    bcomb = (np.asarray(b2, np.float32) @ np.asarray(Wfc, np.float32)
             + np.asarray(bfc, np.float32))
    bcrep = np.ascontiguousarray(np.tile(bcomb[None, :], (P, 1)))

    in_maps = []
    for ci in range(NC):
        in_maps.append({
            "xT": np.ascontiguousarray(xT[ci]),
            "w1": W1, "b1rep": b1rep, "wcombrep": wcombrep, "bcrep": bcrep,
            "dinvc": np.ascontiguousarray(dinvc[ci]),
            "idx16": np.ascontiguousarray(idx16[ci]),
        })
    return in_maps


_NC_CACHE = {}
LAST_RESULTS = None


def make_cfg(x, edge_index):
    n = x.shape[0]
    nreal = -(-n // NC)
    nsh_t = nreal // P + 1          # always >= 1 pad row per shard
    s = max(d for d in range(1, nsh_t + 1) if nsh_t % d == 0 and d <= 7)
    if V2_S and nsh_t % V2_S == 0:
        s = V2_S

    src = np.asarray(edge_index[0], dtype=np.int64)
    dst = np.asarray(edge_index[1], dtype=np.int64)
    esrc = np.concatenate([src, np.arange(n, dtype=np.int64)])
    edst = np.concatenate([dst, np.arange(n, dtype=np.int64)])
    newloc = node_perm(edge_index, n)
    core = edst // nreal
    loc = newloc[edst]
    tl = loc // P
    d_loc = loc % P
    nsh = nsh_t * P
    gsrc = (esrc // nreal) * nsh + newloc[esrc]
    buck = gsrc // (2 * nsh)
    cell = ((core * nsh_t + tl) * NBUCK + buck) * P + d_loc
    cnt = np.bincount(cell, minlength=NC * nsh_t * NBUCK * P)
    caps = cnt.reshape(NC, nsh_t, NBUCK, P).max(axis=(0, 2, 3))
    caps = np.maximum(caps, 1)
    return Cfg(n, caps, s, nsh_t)


def kernel(x, edge_index, W1, b1, W2, b2, Wfc, bfc):
    x = np.asarray(x)
    edge_index = np.asarray(edge_index)
    n = x.shape[0]

    cfg = make_cfg(x, edge_index)
    key = (n, cfg.nsh_t, cfg.s, cfg.caps)
    if key not in _NC_CACHE:
        _NC_CACHE[key] = build_nc(cfg)
    nc = _NC_CACHE[key]

    in_maps = host_prep(x, edge_index, W1, b1, W2, b2, Wfc, bfc, cfg)
    res = run_bass_kernel_spmd(nc, in_maps, core_ids=list(range(NC)))
    global LAST_RESULTS
    LAST_RESULTS = res

    newloc = node_perm(edge_index, n)
    outs = []
    left = n
    for ci in range(NC):
        take = min(cfg.nreal, left)
        r0 = ci * cfg.nreal
        outs.append(res.results[ci]["out"].T[newloc[r0:r0 + take]])
        left -= take
    return np.ascontiguousarray(
        np.concatenate(outs, axis=0)).astype(np.float32)


# revision 4
# speedup vs baseline: 1.1465x; 1.0923x over previous
"""GCN (2x GCNConv + LeakyReLU + Linear) on 8 Trainium2 NeuronCores — v2.

Same sharding/data layout as the baseline kernel (nodes contiguous across 8
cores, edges by destination shard, self-loops as edges, dinv factorization,
unweighted segmented-sum via per-tile XY-reduce over a [p, f, bucket, slot]
access pattern), with the structural fixes found by profiling:

1. dma_gather calls are split to <= MAXCH chunks (~6k indices): calls above
   ~8k indices hit a severe slow path (~12x).
2. Reduce outputs land in one per-layer [128, 98*64] stage; the dinv/bias/
   LeakyReLU post-processing runs as ~5 whole-layer instructions instead of
   ~7 per group (per-instruction overhead on this runtime is 10s of us).
3. Layer-2 tail (z2 @ (W2@Wfc) + b) is computed with broadcast-multiply +
   axis-X reduce on the vector engine instead of 98 PE transposes + 25
   matmuls; dinv and bias are folded in afterwards (dinv commutes with the
   linear map).
"""
import sys
import os

sys.path.insert(0, "/opt/trn_rl_repo")

import numpy as np

import concourse.bass as bass
import concourse.mybir as mybir
import concourse.tile as tile
import concourse.bacc as bacc
from concourse.bass_utils import run_bass_kernel_spmd
from concourse.library_config import mlp

P = 128
NC = 8
NEG = 0.01
NBUCK = 4
# Tuned constants (measured on this runtime):
# - gather calls capped at 47 chunks (~6k indices): >8192-idx calls hit a
#   ~12x slow path; ~1.5k-idx calls drown in per-call overhead
# - single SWDGE queue: 2/4 queues measured slower
MAXCH = 47
NQ = 1
NORED = False
V2_S = 0
V2_BUFS = 1
NEGPAD = False

F32 = mybir.dt.float32
I16 = mybir.dt.int16


class Cfg:
    def __init__(self, n_nodes, caps, s_tiles, nsh_tiles):
        self.n_nodes = n_nodes
        self.d_in = 128
        self.d_mid = 64
        self.d_out = 4
        self.caps = tuple(int(v) for v in caps)   # slots per (tile, bucket)
        self.s = s_tiles
        self.nsh_t = nsh_tiles
        self.nsh = nsh_tiles * P
        self.nreal = -(-n_nodes // NC)
        assert self.nreal < self.nsh, "need at least one pad row per shard"
        self.npad = NC * self.nsh
        self.buck = 2 * self.nsh                  # 8 shards / 4 buckets
        assert self.buck <= 32768
        assert nsh_tiles % s_tiles == 0
        self.ngrp = nsh_tiles // s_tiles
        self.zrow = self.nreal                    # in-bucket index of a zero row
        # per-group call size (chunks) and idx column bases
        self.szg = [sum(self.caps[g * s_tiles:(g + 1) * s_tiles])
                    for g in range(self.ngrp)]
        self.icols_g = [NBUCK * z * 8 for z in self.szg]   # int16 cols per group
        self.ibase = np.concatenate([[0], np.cumsum(self.icols_g)]).astype(int)
        self.icols = int(self.ibase[-1])


def build_nc(cfg: Cfg, outer_loop=1):
    c = cfg
    D = c.d_mid
    S = c.s
    T = c.nsh_t
    nc = bacc.Bacc("TRN2", target_bir_lowering=False, debug=False,
                   num_devices=NC, num_swdge_queues=NQ)
    t_xT = nc.dram_tensor("xT", [c.d_in, c.nsh], F32, kind="ExternalInput")
    t_w1 = nc.dram_tensor("w1", [c.d_in, D], F32, kind="ExternalInput")
    t_b1 = nc.dram_tensor("b1rep", [P, D], F32, kind="ExternalInput")
    t_wc = nc.dram_tensor("wcombrep", [P, c.d_out * D], F32,
                          kind="ExternalInput")
    t_bc = nc.dram_tensor("bcrep", [P, c.d_out], F32, kind="ExternalInput")
    t_dinv = nc.dram_tensor("dinvc", [P, T], F32, kind="ExternalInput")
    t_idx = nc.dram_tensor("idx16", [P, c.icols], I16, kind="ExternalInput")
    t_out = nc.dram_tensor("out", [c.d_out, c.nsh], F32, kind="ExternalOutput")

    with tile.TileContext(nc) as tc:
        with (
            tc.tile_pool(name="const", bufs=1) as cp,
            tc.tile_pool(name="stg", bufs=1) as stp,
            tc.tile_pool(name="sb", bufs=2) as sbp,
            tc.tile_pool(name="ps_h", bufs=2, space="PSUM") as ps_h,
            tc.tile_pool(name="dram", bufs=1, space="DRAM") as dp,
        ):
            nc.gpsimd.load_library(mlp)

            w1_sb = cp.tile([c.d_in, D], F32)
            nc.sync.dma_start(w1_sb[:], t_w1[:])
            b1_sb = cp.tile([P, D], F32)
            nc.sync.dma_start(b1_sb[:], t_b1[:])
            wc_sb = cp.tile([P, c.d_out * D], F32)
            nc.sync.dma_start(wc_sb[:], t_wc[:])
            bc_sb = cp.tile([P, c.d_out], F32)
            nc.sync.dma_start(bc_sb[:], t_bc[:])
            dinv_sb = cp.tile([P, T], F32)
            nc.sync.dma_start(dinv_sb[:], t_dinv[:])

            h_shard = dp.tile([c.nsh, D], F32)
            a_shard = dp.tile([c.nsh, D], F32)
            h_fulls = [dp.tile([c.npad, D], F32, addr_space="Shared",
                               name=f"h_full{r}") for r in range(outer_loop)]
            a_fulls = [dp.tile([c.npad, D], F32, addr_space="Shared",
                               name=f"a_full{r}") for r in range(outer_loop)]

            # ---- phase A: h_shard = (dinv*x) @ W1, row-major for gathers ----
            with tc.tile_pool(name="pa", bufs=1) as pa:
                xT_sb = pa.tile([c.d_in, c.nsh], F32)
                nc.sync.dma_start(xT_sb[:], t_xT[:])
                hstage = pa.tile([P, T * D], F32)
                for i in range(T):
                    ph = ps_h.tile([P, D], F32, space="PSUM", tag="ph")
                    nc.tensor.matmul(out=ph[:],
                                     lhsT=xT_sb[:, i * P:(i + 1) * P],
                                     rhs=w1_sb[:], start=True, stop=True)
                    nc.vector.tensor_copy(hstage[:, i * D:(i + 1) * D], ph[:])
                nc.sync.dma_start(
                    h_shard[:].rearrange("(t p) f -> p t f", p=P),
                    hstage[:].rearrange("p (t f) -> p t f", f=D))

            qctr = [0]

            def msg_pass(table, layer, gxp, stage):
                # gather + per-tile segmented sums into `stage` [P, T*D]
                st3 = stage[:].rearrange("p (t f) -> p t f", f=D)
                for g in range(c.ngrp):
                    szg = c.szg[g]
                    ib = sbp.tile([P, c.icols_g[g]], I16, tag="ib")
                    nc.sync.dma_start(
                        ib[:], t_idx[:, int(c.ibase[g]):int(c.ibase[g + 1])])
                    gb = gxp.tile([P, NBUCK * szg * D], F32, tag="gb")
                    if NEGPAD:
                        nc.vector.memset(gb[:], 0.0)
                    # range-outer / bucket-inner call order: a chunk range's
                    # four bucket slices complete together, so the reduces
                    # for its tiles can overlap the next range's gathers.
                    for c0 in range(0, szg, MAXCH):
                        c1 = min(szg, c0 + MAXCH)
                        for b in range(NBUCK):
                            nc.gpsimd.dma_gather(
                                gb[:, (b * szg + c0) * D:(b * szg + c1) * D]
                                  .rearrange("p (ch f) -> p ch f", ch=c1 - c0),
                                table[b * c.buck:(b + 1) * c.buck, :],
                                ib[:, (b * szg + c0) * 8:(b * szg + c1) * 8],
                                (c1 - c0) * P, (c1 - c0) * P, D,
                                single_packet=False,
                                queue_num=qctr[0] % NQ,
                            )
                            qctr[0] += 1
                    if NORED:
                        nc.vector.memset(stage[:, g * S * D:(g + 1) * S * D],
                                         0.0)
                        continue
                    off = 0
                    for ti in range(S):
                        t = g * S + ti
                        cap = c.caps[t]
                        ap4 = (gb[:]
                               .rearrange("p (b ch f) -> p b ch f", b=NBUCK,
                                          ch=szg)[:, :, off:off + cap, :]
                               .rearrange("p b s f -> p f b s"))
                        nc.vector.reduce_sum(
                            out=stage[:, t * D:(t + 1) * D]
                                .unsqueeze(2).unsqueeze(3),
                            in_=ap4, axis=mybir.AxisListType.XY)
                        off += cap

                dv = dinv_sb[:].unsqueeze(2).broadcast_to([P, T, D])
                if layer == 1:
                    # a = leaky(agg*dinv + b1); table2 rows = dinv*a
                    nc.vector.tensor_tensor(out=st3, in0=st3, in1=dv,
                                            op=mybir.AluOpType.mult)
                    b13 = b1_sb[:].unsqueeze(1).broadcast_to([P, T, D])
                    nc.vector.tensor_tensor(out=st3, in0=st3, in1=b13,
                                            op=mybir.AluOpType.add)
                    nc.scalar.activation(
                        stage[:], stage[:],
                        mybir.ActivationFunctionType.Lrelu, alpha=NEG)
                    nc.vector.tensor_tensor(out=st3, in0=st3, in1=dv,
                                            op=mybir.AluOpType.mult)
                    nc.sync.dma_start(
                        a_shard[:].rearrange("(t p) f -> p t f", p=P),
                        st3)

            for _r in range(outer_loop):
              nc.gpsimd.collective_compute(
                  "AllGather", mybir.AluOpType.bypass,
                  replica_groups=[list(range(NC))],
                  ins=[h_shard.opt()], outs=[h_fulls[_r].opt()],
              )
              with tc.tile_pool(name=f"gx1_{_r}", bufs=V2_BUFS) as gx1:
                stage1 = gx1.tile([P, T * D], F32, name=f"st1_{_r}")
                msg_pass(h_fulls[_r], 1, gx1, stage1)

              nc.gpsimd.collective_compute(
                  "AllGather", mybir.AluOpType.bypass,
                  replica_groups=[list(range(NC))],
                  ins=[a_shard.opt()], outs=[a_fulls[_r].opt()],
              )

              with tc.tile_pool(name=f"gx2_{_r}", bufs=V2_BUFS) as gx2:
                stage2 = gx2.tile([P, T * D], F32, name=f"st2_{_r}")
                msg_pass(a_fulls[_r], 2, gx2, stage2)
                # FC tail: out[o, (t p)] = dinv[n] * (agg2[n] @ Wc)[o] + bc[o]
                st3 = stage2[:].rearrange("p (t f) -> p t f", f=D)
                TH = T // 2
                for h in range(2):
                    t0 = h * TH
                    tmp = gx2.tile([P, TH * c.d_out * D], F32, tag="gb")
                    tmp4 = tmp[:].rearrange("p (t o f) -> p t o f",
                                            o=c.d_out, f=D)
                    in0 = st3[:, t0:t0 + TH, :].unsqueeze(2) \
                        .broadcast_to([P, TH, c.d_out, D])
                    in1 = wc_sb[:].rearrange("p (o f) -> p o f", o=c.d_out) \
                        .unsqueeze(1).broadcast_to([P, TH, c.d_out, D])
                    nc.vector.tensor_tensor(out=tmp4, in0=in0, in1=in1,
                                            op=mybir.AluOpType.mult)
                    red = gx2.tile([P, TH * c.d_out], F32, tag="fcred")
                    red3 = red[:].rearrange("p (t o) -> p t o", o=c.d_out)
                    nc.vector.reduce_sum(out=red3.unsqueeze(3), in_=tmp4,
                                         axis=mybir.AxisListType.X)
                    dv3 = dinv_sb[:, t0:t0 + TH].unsqueeze(2) \
                        .broadcast_to([P, TH, c.d_out])
                    nc.vector.tensor_tensor(out=red3, in0=red3, in1=dv3,
                                            op=mybir.AluOpType.mult)
                    bc3 = bc_sb[:].unsqueeze(1).broadcast_to([P, TH, c.d_out])
                    nc.vector.tensor_tensor(out=red3, in0=red3, in1=bc3,
                                            op=mybir.AluOpType.add)
                    for o in range(c.d_out):
                        nc.sync.dma_start(
                            t_out[o:o + 1, t0 * P:(t0 + TH) * P]
                            .rearrange("one (t p) -> p (one t)", p=P),
                            red3[:, :, o])

    nc.compile()
    return nc


def node_perm(edge_index, n):
    """Per-core degree-sorted local positions: newloc[g] for global node g.

    Sorting each shard's nodes by in-degree makes per-tile slot caps tight
    (tiles hold nearly-equal-degree nodes), cutting gather descriptors.
    """
    dst = np.asarray(edge_index[1], dtype=np.int64)
    deg = np.bincount(np.concatenate([dst, np.arange(n, dtype=np.int64)]),
                      minlength=n)
    nreal = -(-n // NC)
    newloc = np.empty(n, np.int64)
    for ci in range(NC):
        lo, hi = ci * nreal, min(n, (ci + 1) * nreal)
        order = np.argsort(-deg[lo:hi], kind="stable")
        newloc[lo:hi][order] = np.arange(hi - lo)
    return newloc


def host_prep(x, edge_index, W1, b1, W2, b2, Wfc, bfc, cfg: Cfg):
    c = cfg
    n = c.n_nodes
    S = c.s
    src = np.asarray(edge_index[0], dtype=np.int64)
    dst = np.asarray(edge_index[1], dtype=np.int64)

    deg = np.bincount(dst, minlength=n).astype(np.float64) + 1.0
    dinv = (1.0 / np.sqrt(deg)).astype(np.float32)

    esrc = np.concatenate([src, np.arange(n, dtype=np.int64)])
    edst = np.concatenate([dst, np.arange(n, dtype=np.int64)])

    nreal = c.nreal
    newloc = node_perm(edge_index, n)
    core = edst // nreal
    loc = newloc[edst]
    tl = loc // P
    d_loc = loc % P
    gsrc = (esrc // nreal) * c.nsh + newloc[esrc]
    buck = gsrc // c.buck
    inb = (gsrc - buck * c.buck).astype(np.int16)

    # slot rank within each (core, tile, bucket, dst-node) cell
    cell = ((core * c.nsh_t + tl) * NBUCK + buck) * P + d_loc
    order = np.argsort(cell, kind="stable")
    cell_s = cell[order]
    nbins = NC * c.nsh_t * NBUCK * P
    start = np.searchsorted(cell_s, np.arange(nbins))
    rank = np.arange(len(cell_s)) - start[cell_s]
    slot = np.empty(len(cell), np.int64)
    slot[order] = rank

    # edge -> idx16 position
    caps = np.asarray(c.caps, np.int64)
    grp_off = np.concatenate([[0], np.cumsum(caps)])      # global chunk cumsum
    g_of_t = np.arange(c.nsh_t) // S
    grp_start_chunk = grp_off[g_of_t * S]                 # chunks before group
    tioff = grp_off[:c.nsh_t] - grp_start_chunk           # chunk offset in group
    szg_arr = np.asarray(c.szg, np.int64)
    ibase = np.asarray(c.ibase, np.int64)

    g_e = tl // S
    i_in_call = (tioff[tl] + slot) * P + d_loc
    col = ibase[g_e] + buck * szg_arr[g_e] * 8 + i_in_call // 16
    row = i_in_call % 16

    keep = slot < caps[tl]
    if NEGPAD:
        # pads skip descriptor generation; buckets 1-3 only (a -1 from
        # bucket 0's base would address before the table if it *were* read)
        idx16 = np.full((NC, 16, c.icols), -1, np.int16)
        for g in range(c.ngrp):
            b0 = int(ibase[g])
            idx16[:, :, b0:b0 + c.szg[g] * 8] = c.zrow
    else:
        idx16 = np.full((NC, 16, c.icols), c.zrow, np.int16)
    idx16[core[keep], row[keep], col[keep]] = inb[keep]
    idx16 = np.tile(idx16, (1, 8, 1))                     # [NC, 128, icols]

    # dinv-scaled, padded, sharded, transposed x (degree-sorted node order)
    x = np.asarray(x, dtype=np.float32) * dinv[:, None]
    xT = np.zeros((NC, c.d_in, c.nsh), np.float32)
    dinvc = np.zeros((NC, P, c.nsh_t), np.float32)
    for ci in range(NC):
        r0 = ci * nreal
        r1 = min(n, r0 + nreal)
        cols = newloc[r0:r1]
        xT[ci][:, cols] = x[r0:r1].T
        dv = np.zeros(c.nsh, np.float32)
        dv[cols] = dinv[r0:r1]
        dinvc[ci] = dv.reshape(c.nsh_t, P).T

    W1 = np.ascontiguousarray(np.asarray(W1, np.float32))
    b1rep = np.tile(np.asarray(b1, np.float32)[None, :], (P, 1))
    wcomb = (np.asarray(W2, np.float32) @ np.asarray(Wfc, np.float32))
    wcombrep = np.ascontiguousarray(
        np.tile(wcomb.T.reshape(1, -1), (P, 1)))      # [P, (o f)]
    bcomb = (np.asarray(b2, np.float32) @ np.asarray(Wfc, np.float32)
             + np.asarray(bfc, np.float32))
    bcrep = np.ascontiguousarray(np.tile(bcomb[None, :], (P, 1)))

    in_maps = []
    for ci in range(NC):
        in_maps.append({
            "xT": np.ascontiguousarray(xT[ci]),
            "w1": W1, "b1rep": b1rep, "wcombrep": wcombrep, "bcrep": bcrep,
            "dinvc": np.ascontiguousarray(dinvc[ci]),
            "idx16": np.ascontiguousarray(idx16[ci]),
        })
    return in_maps


_NC_CACHE = {}
LAST_RESULTS = None


def make_cfg(x, edge_index):
    n = x.shape[0]
    nreal = -(-n // NC)
    nsh_t = nreal // P + 1          # always >= 1 pad row per shard
    s = max(d for d in range(1, nsh_t + 1) if nsh_t % d == 0 and d <= 7)
    if V2_S and nsh_t % V2_S == 0:
        s = V2_S

    src = np.asarray(edge_index[0], dtype=np.int64)
    dst = np.asarray(edge_index[1], dtype=np.int64)
    esrc = np.concatenate([src, np.arange(n, dtype=np.int64)])
    edst = np.concatenate([dst, np.arange(n, dtype=np.int64)])
    newloc = node_perm(edge_index, n)
    core = edst // nreal
    loc = newloc[edst]
    tl = loc // P
    d_loc = loc % P
    nsh = nsh_t * P
    gsrc = (esrc // nreal) * nsh + newloc[esrc]
    buck = gsrc // (2 * nsh)
    cell = ((core * nsh_t + tl) * NBUCK + buck) * P + d_loc
    cnt = np.bincount(cell, minlength=NC * nsh_t * NBUCK * P)
    caps = cnt.reshape(NC, nsh_t, NBUCK, P).max(axis=(0, 2, 3))
    caps = np.maximum(caps, 1)
    return Cfg(n, caps, s, nsh_t)


def kernel(x, edge_index, W1, b1, W2, b2, Wfc, bfc):
    x = np.asarray(x)
    edge_index = np.asarray(edge_index)
    n = x.shape[0]

    cfg = make_cfg(x, edge_index)
    key = (n, cfg.nsh_t, cfg.s, cfg.caps)
    if key not in _NC_CACHE:
        _NC_CACHE[key] = build_nc(cfg)
    nc = _NC_CACHE[key]

    in_maps = host_prep(x, edge_index, W1, b1, W2, b2, Wfc, bfc, cfg)
    res = run_bass_kernel_spmd(nc, in_maps, core_ids=list(range(NC)))
    global LAST_RESULTS
    LAST_RESULTS = res

    newloc = node_perm(edge_index, n)
    outs = []
    left = n
    for ci in range(NC):
        take = min(cfg.nreal, left)
        r0 = ci * cfg.nreal
        outs.append(res.results[ci]["out"].T[newloc[r0:r0 + take]])
        left -= take
    return np.ascontiguousarray(
        np.concatenate(outs, axis=0)).astype(np.float32)
